# revision 9
# baseline (speedup 1.0000x reference)
"""Trainium2 Bass kernel for nn_ConjunctionLayer (fuzzy-logic AND layer).

out[b, n] = prod_d (1 - (1 - x[b,d]) * W[n,d])

Reformulation: with u = 1-x (in [0,1]) and w = W (in [0, 0.1)), z = u*w in
[0, 0.1), so

    log out[b,n] = sum_d log(1 - z_bdn)  ~=  -sum_{k=1..3} a_k * sum_d u^k w^k

where each inner sum over d is a matmul of elementwise powers.  a_k are
least-squares Chebyshev-node coefficients of -log(1-z)/z on [0, 0.1]
(per-element approx error < 1e-6).

    out = exp(-(a_1 * u@w.T + a_2 * u^2@(w^2).T + a_3 * u^3@(w^3).T))

All three matmul groups run as float32r (PE replicated-fp32: full rate at
N>=256, ~7e-5 product precision — measured), so no bf16 casts or hi/lo
splits are needed; elementwise powers stay in fp32.

Sharding: data-parallel over batch. 8 cores x 128 batch rows each; W
replicated. Inputs are transposed host-side (layout prep while sharding) so
the contraction dim d lands on SBUF partitions with zero on-device
transposes.
"""

import numpy as np

import concourse.bacc as bacc
import concourse.bass as bass
import concourse.mybir as mybir
import concourse.tile as tile
from concourse.alu_op_type import AluOpType
from concourse.bass_utils import run_bass_kernel_spmd

B, D, N = 1024, 512, 512
NCORES = 8
BS = B // NCORES          # batch rows per core
KC = D // 128             # contraction chunks of 128

# Degree-3 fit of -log(1-z)/z on [0, 0.1] (see numerics_check.py)
A1 = 1.00000904
A2 = 0.49839935
A3 = 0.37467614

FP32 = mybir.dt.float32
FP32R = mybir.dt.float32r


def _emit(ctx, tc, nc, xT_d, wT_d, o_d):
    pool = ctx.enter_context(tc.tile_pool(name="sbuf", bufs=1))
    psum = ctx.enter_context(tc.tile_pool(name="psum", bufs=1, space="PSUM"))
    Act = mybir.ActivationFunctionType

    # Warm the exp activation table while DMAs run.
    warm = pool.tile([128, 1], FP32)
    nc.vector.memset(warm, 0.0)
    nc.scalar.activation(warm, warm, Act.Exp)

    # ---- loads (d on partitions) ----
    # xTs[p, kc, b] = x[b, kc*128+p]
    xTs = pool.tile([128, KC, BS], FP32)
    nc.sync.dma_start(xTs, xT_d.rearrange("(kc p) b -> p kc b", p=128))
    wTs = []                # wTs[kc][p, n] = W[n, kc*128+p]  (fp32r-tagged)
    for kc in range(KC):
        t = pool.tile([128, N], FP32R, name=f"wT{kc}")
        eng = nc.scalar if kc % 2 else nc.sync   # spread over both HWDGE rings
        eng.dma_start(t, wT_d[kc * 128:(kc + 1) * 128, :].bitcast(FP32R))
        wTs.append(t)

    # ---- u-side elementwise (3 DVE ops, coefficient ratios folded in) ----
    t1 = pool.tile([128, KC, BS], FP32R)    # a1*u = -a1*x + a1
    nc.vector.tensor_scalar(t1, xTs, -A1, A1, AluOpType.mult, AluOpType.add)
    u2s = pool.tile([128, KC, BS], FP32R)   # a2*u^2 = (t1 * a2/a1^2) * t1
    nc.vector.scalar_tensor_tensor(u2s, t1, A2 / (A1 * A1), t1,
                                   AluOpType.mult, AluOpType.mult)
    u3s = pool.tile([128, KC, BS], FP32R)   # a3*u^3 = (u2s * a3/(a2*a1)) * t1
    nc.vector.scalar_tensor_tensor(u3s, u2s, A3 / (A2 * A1), t1,
                                   AluOpType.mult, AluOpType.mult)

    # ---- w-side elementwise (fp32, per kc chunk [128, 512]) ----
    w2s, w3s = [], []
    for kc in range(KC):
        w2 = pool.tile([128, N], FP32R, name=f"w2{kc}")
        nc.scalar.activation(w2, wTs[kc], Act.Square)   # ACT: w^2
        w3 = pool.tile([128, N], FP32R, name=f"w3{kc}")
        nc.vector.tensor_mul(w3, w2, wTs[kc])           # DVE: w^3
        w2s.append(w2)
        w3s.append(w3)

    # ---- float32r matmul accumulation: S[b, n] in one PSUM bank ----
    # Pass-major order: k=1 operands are ready as DMA chunks land; the
    # w^2/w^3 chains fill in behind.
    ps_out = psum.tile([128, N], FP32, name="ps_out")
    mms = []
    for us, ws in [(t1, wTs), (u2s, w2s), (u3s, w3s)]:
        for kc in range(KC):
            mms.append((us[:, kc, :], ws[kc]))
    for i, (ut, wt) in enumerate(mms):
        nc.tensor.matmul(ps_out, ut, wt,
                         start=(i == 0), stop=(i == len(mms) - 1))

    # ---- out = exp(-S) ----
    outs = pool.tile([128, N], FP32)
    nc.scalar.activation(outs, ps_out, Act.Exp, scale=-1.0)
    nc.sync.dma_start(o_d, outs)


_CACHE = {}


def _build():
    if "nc" in _CACHE:
        return _CACHE["nc"]
    nc = bacc.Bacc("TRN2", target_bir_lowering=False, debug=False,
                   num_devices=NCORES)
    xT_d = nc.dram_tensor("xT", [D, BS], FP32, kind="ExternalInput").ap()
    wT_d = nc.dram_tensor("wT", [D, N], FP32, kind="ExternalInput").ap()
    o_d = nc.dram_tensor("out", [BS, N], FP32, kind="ExternalOutput").ap()
    from contextlib import ExitStack
    with tile.TileContext(nc) as tc, ExitStack() as ctx:
        _emit(ctx, tc, nc, xT_d, wT_d, o_d)
    nc.compile()
    _CACHE["nc"] = nc
    return nc


def kernel(x: np.ndarray, W: np.ndarray) -> np.ndarray:
    nc = _build()
    x = np.asarray(x, np.float32)
    W = np.asarray(W, np.float32)
    xT = np.ascontiguousarray(x.T)            # [D, B]
    wT = np.ascontiguousarray(W.T)            # [D, N]
    in_maps = [{"xT": np.ascontiguousarray(xT[:, i * BS:(i + 1) * BS]),
                "wT": wT} for i in range(NCORES)]
    res = run_bass_kernel_spmd(nc, in_maps, list(range(NCORES)))
    return np.concatenate([res.results[i]["out"] for i in range(NCORES)], axis=0)


# revision 12
# speedup vs baseline: 1.1155x; 1.1155x over previous
"""Trainium2 Bass kernel for nn_ConjunctionLayer (fuzzy-logic AND layer).

out[b, n] = prod_d (1 - (1 - x[b,d]) * W[n,d])

Reformulation: with u = 1-x (in [0,1]) and w = W (in [0, 0.1)), z = u*w in
[0, 0.1), so

    log out[b,n] = sum_d log(1 - z_bdn)  ~=  -sum_{k=1..3} a_k * sum_d u^k w^k

where each inner sum over d is a matmul of elementwise powers.  a_k are
least-squares Chebyshev-node coefficients of -log(1-z)/z on [0, 0.1]
(per-element approx error < 1e-6).

    out = exp(-(a_1 * u@w.T + a_2 * u^2@(w^2).T + a_3 * u^3@(w^3).T))

All three matmul groups run as float32r (PE replicated-fp32: full rate at
N>=256, ~7e-5 product precision — measured), so no bf16 casts or hi/lo
splits are needed; elementwise powers stay in fp32.

Sharding: data-parallel over batch. 8 cores x 128 batch rows each; W
replicated. Inputs are transposed host-side (layout prep while sharding) so
the contraction dim d lands on SBUF partitions with zero on-device
transposes.
"""

import numpy as np

import concourse.bacc as bacc
import concourse.bass as bass
import concourse.mybir as mybir
import concourse.tile as tile
from concourse.alu_op_type import AluOpType
from concourse.bass_utils import run_bass_kernel_spmd

B, D, N = 1024, 512, 512
NCORES = 8
BS = B // NCORES          # batch rows per core
KC = D // 128             # contraction chunks of 128

# Degree-3 fit of -log(1-z)/z on [0, 0.1] (see numerics_check.py)
A1 = 1.00000904
A2 = 0.49839935
A3 = 0.37467614

FP32 = mybir.dt.float32
FP32R = mybir.dt.float32r


def _emit(ctx, tc, nc, xT_d, wT_d, o_d):
    pool = ctx.enter_context(tc.tile_pool(name="sbuf", bufs=1))
    psum = ctx.enter_context(tc.tile_pool(name="psum", bufs=1, space="PSUM"))
    Act = mybir.ActivationFunctionType

    # Warm the exp activation table while DMAs run.
    warm = pool.tile([128, 1], FP32)
    nc.vector.memset(warm, 0.0)
    nc.scalar.activation(warm, warm, Act.Exp)

    # ---- loads (d on partitions) ----
    # xTs[p, kc, b] = x[b, kc*128+p]
    xTs = pool.tile([128, KC, BS], FP32)
    nc.sync.dma_start(xTs, xT_d.rearrange("(kc p) b -> p kc b", p=128))
    wTs = []                # wTs[kc][p, n] = W[n, kc*128+p]  (fp32r-tagged)
    for kc in range(KC):
        t = pool.tile([128, N], FP32R, name=f"wT{kc}")
        eng = nc.scalar if kc % 2 else nc.sync   # spread over both HWDGE rings
        eng.dma_start(t, wT_d[kc * 128:(kc + 1) * 128, :].bitcast(FP32R))
        wTs.append(t)

    # ---- u-side elementwise (3 DVE ops, coefficient ratios folded in) ----
    t1 = pool.tile([128, KC, BS], FP32R)    # a1*u = -a1*x + a1
    nc.vector.tensor_scalar(t1, xTs, -A1, A1, AluOpType.mult, AluOpType.add)
    u2s = pool.tile([128, KC, BS], FP32R)   # a2*u^2 = (t1 * a2/a1^2) * t1
    nc.vector.scalar_tensor_tensor(u2s, t1, A2 / (A1 * A1), t1,
                                   AluOpType.mult, AluOpType.mult)
    u3s = pool.tile([128, KC, BS], FP32R)   # a3*u^3 = (u2s * a3/(a2*a1)) * t1
    nc.vector.scalar_tensor_tensor(u3s, u2s, A3 / (A2 * A1), t1,
                                   AluOpType.mult, AluOpType.mult)

    # ---- w-side elementwise (fp32, per kc chunk [128, 512]) ----
    w2s, w3s = [], []
    for kc in range(KC):
        w2 = pool.tile([128, N], FP32R, name=f"w2{kc}")
        nc.scalar.activation(w2, wTs[kc], Act.Square)   # ACT: w^2
        w3 = pool.tile([128, N], FP32R, name=f"w3{kc}")
        nc.vector.tensor_mul(w3, w2, wTs[kc])           # DVE: w^3
        w2s.append(w2)
        w3s.append(w3)

    # ---- float32r matmul accumulation: S[b, n] in one PSUM bank ----
    # Pass-major order: k=1 operands are ready as DMA chunks land; the
    # w^2/w^3 chains fill in behind.
    ps_out = psum.tile([128, N], FP32, name="ps_out")
    mms = []
    for us, ws in [(t1, wTs), (u2s, w2s), (u3s, w3s)]:
        for kc in range(KC):
            mms.append((us[:, kc, :], ws[kc]))
    for i, (ut, wt) in enumerate(mms):
        nc.tensor.matmul(ps_out, ut, wt,
                         start=(i == 0), stop=(i == len(mms) - 1))

    # ---- out = exp(-S), halves; store issued from the scalar engine so the
    # exp -> store dependency rides the ACT FIFO (no cross-engine sem hop) ----
    outs = pool.tile([128, N], FP32)
    NH = N // 2
    for h in range(2):
        sl = slice(h * NH, (h + 1) * NH)
        nc.scalar.activation(outs[:, sl], ps_out[:, sl], Act.Exp, scale=-1.0)
        nc.scalar.dma_start(o_d[:, sl], outs[:, sl])


_CACHE = {}


def _build():
    if "nc" in _CACHE:
        return _CACHE["nc"]
    nc = bacc.Bacc("TRN2", target_bir_lowering=False, debug=False,
                   num_devices=NCORES)
    xT_d = nc.dram_tensor("xT", [D, BS], FP32, kind="ExternalInput").ap()
    wT_d = nc.dram_tensor("wT", [D, N], FP32, kind="ExternalInput").ap()
    o_d = nc.dram_tensor("out", [BS, N], FP32, kind="ExternalOutput").ap()
    from contextlib import ExitStack
    with tile.TileContext(nc) as tc, ExitStack() as ctx:
        _emit(ctx, tc, nc, xT_d, wT_d, o_d)
    nc.compile()
    _CACHE["nc"] = nc
    return nc


def kernel(x: np.ndarray, W: np.ndarray) -> np.ndarray:
    nc = _build()
    x = np.asarray(x, np.float32)
    W = np.asarray(W, np.float32)
    xT = np.ascontiguousarray(x.T)            # [D, B]
    wT = np.ascontiguousarray(W.T)            # [D, N]
    in_maps = [{"xT": np.ascontiguousarray(xT[:, i * BS:(i + 1) * BS]),
                "wT": wT} for i in range(NCORES)]
    res = run_bass_kernel_spmd(nc, in_maps, list(range(NCORES)))
    return np.concatenate([res.results[i]["out"] for i in range(NCORES)], axis=0)
